# revision 24
# baseline (speedup 1.0000x reference)
"""MultiHeadAttention (softmax over heads) Trainium2 kernel.

Math (per batch b):
  k[t, o]   = value[b, t, :] @ conv_w[o, :, 0] + conv_b[o]
  s[h, q, t] = sum_d query[b, q, h*64+d] * k[t, h*64+d]
  w[h, q, t] = softmax over h of (s/8)              (legacy implicit dim=1)
  out[b, q, h*64+d] = sum_t w[h, q, t] * value[b, t, d]

Key identity: s[h,q,t] = qW[h,q,:] . value[t,:] + qb[h,q] where
  qW[h,q,i] = sum_d query[q, h*64+d] * conv_w[h*64+d, i]
so all 8 heads' scores share the same rhs (value) with contraction over the
64 value features (+1 bias row), and k is never materialized.

Layout: scores live t-on-partitions as s[t, h, q] so the h-softmax is a
free-dim segmented reduction and stage B (ctxT[d, (h,q)] += v.T w) consumes
the weights directly. Final PE transposes produce out[q, h*64+d].

Engine budget per core (cost model): ACT = exp only (~66us, the wall),
DVE = t1 + recip + mult + psum copies, Pool = t2 + dsum + r-cast,
PE = scores + stage B + transposes (~60us).

The main loop is a single flattened chunk pipeline across q-blocks: score
matmuls of block N+1 are issued before the deferred stage-B batch of block
N so the ACT exp stream never starves at block boundaries. The final group
is emitted per-chunk to shrink the serial tail.

Sharding: data-parallel over (batch, query-rows): core c handles batch c//4,
query rows (c%4)*512 ... +512. No collectives.
"""

import sys

sys.path.insert(0, "/opt/trn_rl_repo")

import numpy as np
import ml_dtypes

import concourse.bass as bass
import concourse.bacc as bacc
import concourse.tile as tile
from concourse import mybir
from concourse.bass_utils import run_bass_kernel_spmd
from concourse.masks import make_identity

N_CORES = 8
B, TQ, TV, D, H, DH = 2, 2048, 2048, 512, 8, 64
QPC = (B * TQ) // N_CORES  # 512 query rows per core
QB = 128                   # q-block
NQB = QPC // QB            # 4
TCH = 128                  # t-chunk
NTC = TV // TCH            # 16
GRP = 4                    # t-chunks per softmax batch
NGRP = NTC // GRP

F32 = mybir.dt.float32
F32R = mybir.dt.float32r
BF16 = mybir.dt.bfloat16
SA_F32R = False  # score path dtype (False -> bf16)

_CACHE = {}
BUILD_CFG = dict()


def build_nc(reps=1, grp=GRP, eb=4, tb=4, db=6, wb=4, sb=3,
             sa_f32r=None, tail_g=2, taper=True, split_tail=False,
             gp_t2=True, gp_dsum=True, gp_cast=True, gp_t1=True,
             qw_dve=False, mult_splits=2, t1_dve=2):
    if sa_f32r is None:
        sa_f32r = SA_F32R
    SDT = F32R if sa_f32r else BF16     # score-path storage/matmul dtype
    nc = bacc.Bacc("TRN2", target_bir_lowering=False, debug=False,
                   num_devices=N_CORES)

    # Per-core inputs (host-prepped layouts).
    # qTp[d, h, q] = query[q, h*64+d]
    qTp = nc.dram_tensor("qTp", [DH, H * QPC], SDT, kind="ExternalInput")
    # vT1[i, t] = value[t, i] for i<64; row 64 = ones
    vT1 = nc.dram_tensor("vT1", [DH + 1, TV], SDT, kind="ExternalInput")
    # vP[p, c, d] = value[c*128 + p, d]
    vP = nc.dram_tensor("vP", [TCH, NTC * DH], BF16, kind="ExternalInput")
    # wA[d, h, i] = conv_w[h*64+d, i, 0] (i<64); wA[d, h, 64] = conv_b[h*64+d]
    wA = nc.dram_tensor("wA", [DH, H * (DH + 1)], SDT, kind="ExternalInput")
    out = nc.dram_tensor("out", [QPC, D], F32, kind="ExternalOutput")

    ngrp = NTC // grp

    with tile.TileContext(nc) as tc:
        with (
            tc.tile_pool(name="consts", bufs=1) as consts,
            tc.tile_pool(name="qwt", bufs=1) as qwt_pool,
            tc.tile_pool(name="vals", bufs=1) as vals,
        ):
            # ---- constants / weights ----
            w_sb = consts.tile([DH, H, DH + 1], SDT)
            nc.sync.dma_start(out=w_sb, in_=wA.rearrange("d (h i) -> d h i", h=H))

            vT_sb = vals.tile([DH + 1, TV], SDT)
            nc.gpsimd.dma_start(out=vT_sb, in_=vT1[:, :])

            ident = consts.tile([2 * DH, 2 * DH], F32)
            make_identity(nc, ident)

            # ---- qWT' computation: [65, H, QPC] in two h-halves ----
            # per-h input DMAs spread over queues so qwt matmuls start early
            qTp3 = qTp.rearrange("d (h q) -> d h q", h=H)
            qwt_halves = []
            dma_engs = [nc.sync, nc.gpsimd, nc.scalar, nc.sync]
            with (
                tc.tile_pool(name="qt_in", bufs=2) as qt_in,
                tc.tile_pool(name="qw_ps", bufs=2, space="PSUM") as qw_ps,
            ):
                for half in (0, 1):
                    qt_sb = qt_in.tile([DH, 4, QPC], SDT)
                    for pair in (0, 1):
                        eng = dma_engs[(half * 2 + pair) % 4]
                        eng.dma_start(
                            out=qt_sb[:, pair * 2:pair * 2 + 2, :],
                            in_=qTp3[:, half * 4 + pair * 2:
                                     half * 4 + pair * 2 + 2, :])
                    qwt_h = qwt_pool.tile([DH + 1, 4, QPC], SDT,
                                          tag=f"qwt{half}")
                    for hh in range(4):
                        h = half * 4 + hh
                        ps = qw_ps.tile([DH + 1, QPC], F32)
                        nc.tensor.matmul(ps, lhsT=w_sb[:, h, :],
                                         rhs=qt_sb[:, hh, :],
                                         start=True, stop=True)
                        # alternate engines so the preamble copies pipeline
                        if (hh + half) % 2 == (0 if qw_dve else 1):
                            nc.vector.tensor_copy(qwt_h[:, hh, :], ps)
                        else:
                            nc.scalar.copy(qwt_h[:, hh, :], ps)
                    qwt_halves.append(qwt_h)

            v_sb = vals.tile([TCH, NTC, DH], BF16)
            nc.gpsimd.dma_start(out=v_sb,
                                in_=vP.rearrange("p (c d) -> p c d", c=NTC))

            # ---- main loop: flattened chunk pipeline ----
            with (
                tc.tile_pool(name="s_ps", bufs=sb, space="PSUM") as s_ps_pool,
                tc.tile_pool(name="ctx_ps", bufs=2, space="PSUM") as ctx_ps_pool,
                tc.tile_pool(name="e_sb", bufs=eb) as e_pool,
                tc.tile_pool(name="tr_sb", bufs=tb) as tr_pool,
                tc.tile_pool(name="d_sb", bufs=db) as d_pool,
                tc.tile_pool(name="w_sb2", bufs=wb) as wt_pool,
                tc.tile_pool(name="ctx_sb", bufs=2) as ctx_sb_pool,
            ):
                import contextlib
                rep_ctx = tc.For_i(0, reps, 1) if reps > 1 else contextlib.nullcontext()
                with rep_ctx:
                    ctx_tiles = {}

                    def emit_scores(qb, g, ci, e):
                        """score matmuls + exp for chunk (qb, g, ci)"""
                        q0 = qb * QB
                        t = g * grp + ci
                        s_ps = s_ps_pool.tile([TCH, H, QB], F32, tag="s")
                        lhsT = vT_sb[:, t * TCH:(t + 1) * TCH]
                        nc.tensor.matmul(
                            s_ps[:, 0:4, :], lhsT=lhsT,
                            rhs=qwt_halves[0][:, :, q0:q0 + QB],
                            start=True, stop=True)
                        nc.tensor.matmul(
                            s_ps[:, 4:8, :], lhsT=lhsT,
                            rhs=qwt_halves[1][:, :, q0:q0 + QB],
                            start=True, stop=True)
                        nc.scalar.activation(
                            out=e[:, ci, :, :], in_=s_ps,
                            func=mybir.ActivationFunctionType.Exp,
                            scale=0.125)
                        return s_ps

                    def emit_softmax(e, wt, cis):
                        """normalize chunks cis (a slice) of group tiles"""
                        t1 = tr_pool.tile([TCH, grp, 4, QB], BF16, tag="t1")
                        # split the first reduction level between DVE and Pool
                        k = t1_dve
                        if k:
                            nc.vector.tensor_add(
                                t1[:, cis, 0:k], e[:, cis, 0:k, :],
                                e[:, cis, 4:4 + k, :])
                        if k < 4:
                            (nc.gpsimd if gp_t1 else nc.vector).tensor_add(
                                t1[:, cis, k:4], e[:, cis, k:4, :],
                                e[:, cis, 4 + k:8, :])
                        t2 = tr_pool.tile([TCH, grp, 2, QB], BF16, tag="t2")
                        (nc.gpsimd if gp_t2 else nc.vector).tensor_add(
                            t2[:, cis], t1[:, cis, 0:2, :], t1[:, cis, 2:4, :])
                        dsum = d_pool.tile([TCH, grp, QB], F32, tag="dsum")
                        (nc.gpsimd if gp_dsum else nc.vector).tensor_add(
                            dsum[:, cis], t2[:, cis, 0, :], t2[:, cis, 1, :])
                        r32 = d_pool.tile([TCH, grp, QB], F32, tag="r32")
                        nc.vector.reciprocal_approx_fast(out=r32[:, cis],
                                                         in_=dsum[:, cis])
                        r16 = d_pool.tile([TCH, grp, QB], BF16, tag="r16")
                        (nc.gpsimd if gp_cast else nc.vector).tensor_copy(
                            r16[:, cis], r32[:, cis])
                        n = cis.stop - cis.start
                        nsp = mult_splits if n > 1 else 1
                        step = max(1, n // nsp)
                        for c0 in range(cis.start, cis.stop, step):
                            cs = slice(c0, min(c0 + step, cis.stop))
                            rs = r16[:, cs]
                            r_bcast = bass.AP(
                                tensor=rs.tensor, offset=rs.offset,
                                ap=[rs.ap[0], rs.ap[1], [0, H], rs.ap[2]],
                            )
                            nc.vector.tensor_mul(wt[:, cs], e[:, cs], r_bcast)

                    def emit_stage_b(qb, g, wt, cis):
                        ctx_ps = ctx_tiles[qb]
                        for ci in range(cis.start, cis.stop):
                            t = g * grp + ci
                            for sub in (0, 1):
                                nc.tensor.matmul(
                                    ctx_ps[sub * DH:(sub + 1) * DH, :, :],
                                    lhsT=v_sb[:, t, :],
                                    rhs=wt[:, ci, :,
                                           sub * (QB // 2):(sub + 1) * (QB // 2)],
                                    start=(t == 0), stop=(t == NTC - 1),
                                    tile_position=(0, sub * DH),
                                    skip_group_check=True)

                    def emit_tail(qb, last=False):
                        q0_ = qb * QB
                        ctx_ps = ctx_tiles.pop(qb)
                        ctx_sb = ctx_sb_pool.tile([2 * DH, H, QB // 2], F32)
                        subs = ((0,), (1,)) if split_tail else ((0, 1),)
                        o_ps = s_ps_pool.tile([DH, 2, H, DH], F32, tag="s")
                        o_sb = ctx_sb_pool.tile([DH, 2, H, DH], F32,
                                                tag="o_sb")

                        def cp(dst, src):
                            # the last tail runs after the final exp: ACT is
                            # idle there while DVE still drains the backlog
                            if last:
                                nc.scalar.copy(dst, src)
                            else:
                                nc.vector.tensor_copy(dst, src)

                        dma_q = (nc.sync, nc.gpsimd)
                        for subset in subs:
                            for sub in subset:
                                cp(ctx_sb[sub * DH:(sub + 1) * DH],
                                   ctx_ps[sub * DH:(sub + 1) * DH])
                            for sub in subset:
                                for h in range(H):
                                    nc.tensor.transpose(
                                        o_ps[:, sub, h, :],
                                        ctx_sb[sub * DH:(sub + 1) * DH, h, :],
                                        ident[sub * DH:(sub + 1) * DH,
                                              sub * DH:(sub + 1) * DH])
                            for sub in subset:
                                cp(o_sb[:, sub], o_ps[:, sub])
                                sub_ap = bass.AP(
                                    tensor=out, offset=q0_ * D + sub * DH * D,
                                    ap=[[D, DH], [1, D]],
                                )
                                dma_q[sub].dma_start(out=sub_ap,
                                                     in_=o_sb[:, sub])

                    # job list: (qb, g) pairs in order
                    jobs = [(qb, g) for qb in range(NQB) for g in range(ngrp)]
                    pending_b = None   # (qb, g, wt, slice) awaiting stage B
                    pending_tail = None
                    for j, (qb, g) in enumerate(jobs):
                        if qb not in ctx_tiles:
                            ctx_tiles[qb] = ctx_ps_pool.tile(
                                [2 * DH, H, QB // 2], F32, tag="ctx",
                                name=f"ctx{qb}")
                        e = e_pool.tile([TCH, grp, H, QB], BF16)
                        wt = wt_pool.tile([TCH, grp, H, QB], BF16)
                        last_job = j == len(jobs) - 1
                        if taper and last_job:
                            # fine-grained emission to shrink the serial tail
                            if pending_b is not None:
                                emit_stage_b(*pending_b)
                                pending_b = None
                            ts = taper if isinstance(taper, int) and taper > 1 else 1
                            for c0 in range(0, grp, ts):
                                cs = slice(c0, min(c0 + ts, grp))
                                for ci in range(cs.start, cs.stop):
                                    emit_scores(qb, g, ci, e)
                                emit_softmax(e, wt, cs)
                                emit_stage_b(qb, g, wt, cs)
                        else:
                            for ci in range(grp):
                                emit_scores(qb, g, ci, e)
                            if pending_tail is not None and g == tail_g:
                                emit_tail(pending_tail)
                                pending_tail = None
                            if pending_b is not None:
                                emit_stage_b(*pending_b)
                                pending_b = None
                            emit_softmax(e, wt, slice(0, grp))
                            pending_b = (qb, g, wt, slice(0, grp))
                        if g == ngrp - 1:
                            if pending_tail is not None:
                                emit_tail(pending_tail)
                            pending_tail = qb
                    if pending_b is not None:
                        emit_stage_b(*pending_b)
                    if pending_tail is not None:
                        emit_tail(pending_tail, last=True)
    nc.compile()
    return nc


def _prep_inputs(query, value, conv_w, conv_b):
    """Build the 8 per-core input maps (host-side sharding + layout)."""
    sdt = np.float32 if SA_F32R else ml_dtypes.bfloat16
    W = conv_w[:, :, 0]  # [512, 64]
    w_aug = np.zeros((DH, H, DH + 1), np.float32)
    w_aug[:, :, :DH] = W.reshape(H, DH, DH).transpose(1, 0, 2)
    w_aug[:, :, DH] = conv_b.reshape(H, DH).T
    w_aug = w_aug.reshape(DH, H * (DH + 1)).astype(sdt)

    per_batch = {}
    for b in range(B):
        vT1 = np.concatenate(
            [value[b].T, np.ones((1, TV), np.float32)], axis=0
        ).astype(sdt)
        vPb = np.ascontiguousarray(
            value[b].reshape(NTC, TCH, DH).transpose(1, 0, 2)
        ).reshape(TCH, NTC * DH).astype(ml_dtypes.bfloat16)
        per_batch[b] = (np.ascontiguousarray(vT1), np.ascontiguousarray(vPb))

    in_maps = []
    for c in range(N_CORES):
        b = c // (N_CORES // B)
        qs = (c % (N_CORES // B)) * QPC
        # qTp[d, h, q] = query[b, qs+q, h*64+d]
        qTp = np.ascontiguousarray(
            query[b, qs:qs + QPC, :].reshape(QPC, H, DH).transpose(2, 1, 0)
        ).reshape(DH, H * QPC).astype(sdt)
        vT1, vPb = per_batch[b]
        in_maps.append({
            "qTp": np.ascontiguousarray(qTp),
            "vT1": vT1,
            "vP": vPb,
            "wA": w_aug,
        })
    return in_maps


def kernel(query, value, conv_w, conv_b, trace=False, **bench_kwargs):
    query = np.asarray(query, np.float32)
    value = np.asarray(value, np.float32)
    conv_w = np.asarray(conv_w, np.float32)
    conv_b = np.asarray(conv_b, np.float32)

    if "nc" not in _CACHE:
        _CACHE["nc"] = build_nc(**BUILD_CFG)
    nc = _CACHE["nc"]

    in_maps = _prep_inputs(query, value, conv_w, conv_b)
    res = run_bass_kernel_spmd(nc, in_maps, core_ids=list(range(N_CORES)),
                               trace=trace, **bench_kwargs)

    out = np.empty((B, TQ, D), np.float32)
    for c in range(N_CORES):
        b = c // (N_CORES // B)
        qs = (c % (N_CORES // B)) * QPC
        out[b, qs:qs + QPC, :] = res.results[c]["out"]
    if trace:
        return out, res
    return out


# revision 25
# speedup vs baseline: 1.5588x; 1.5588x over previous
"""MultiHeadAttention (softmax over heads) Trainium2 kernel.

Math (per batch b):
  k[t, o]   = value[b, t, :] @ conv_w[o, :, 0] + conv_b[o]
  s[h, q, t] = sum_d query[b, q, h*64+d] * k[t, h*64+d]
  w[h, q, t] = softmax over h of (s/8)              (legacy implicit dim=1)
  out[b, q, h*64+d] = sum_t w[h, q, t] * value[b, t, d]

Key identity: s[h,q,t] = qW[h,q,:] . value[t,:] + qb[h,q] where
  qW[h,q,i] = sum_d query[q, h*64+d] * conv_w[h*64+d, i]
so all 8 heads' scores share the same rhs (value) with contraction over the
64 value features (+1 bias row), and k is never materialized.

Layout: scores live t-on-partitions as s[t, h, q] so the h-softmax is a
free-dim segmented reduction and stage B (ctxT[d, (h,q)] += v.T w) consumes
the weights directly. Final PE transposes produce out[q, h*64+d].

Engine budget per core (cost model): ACT = exp only (~66us, the wall),
DVE = t1 + recip + mult + psum copies, Pool = t2 + dsum + r-cast,
PE = scores + stage B + transposes (~60us).

The main loop is a single flattened chunk pipeline across q-blocks: score
matmuls of block N+1 are issued before the deferred stage-B batch of block
N so the ACT exp stream never starves at block boundaries. The final group
is emitted per-chunk to shrink the serial tail.

Sharding: data-parallel over (batch, query-rows): core c handles batch c//4,
query rows (c%4)*512 ... +512. No collectives.
"""

import sys

sys.path.insert(0, "/opt/trn_rl_repo")

import numpy as np
import ml_dtypes

import concourse.bass as bass
import concourse.bacc as bacc
import concourse.tile as tile
from concourse import mybir
from concourse.bass_utils import run_bass_kernel_spmd
from concourse.masks import make_identity

N_CORES = 8
B, TQ, TV, D, H, DH = 2, 2048, 2048, 512, 8, 64
QPC = (B * TQ) // N_CORES  # 512 query rows per core
QB = 128                   # q-block
NQB = QPC // QB            # 4
TCH = 128                  # t-chunk
NTC = TV // TCH            # 16
GRP = 4                    # t-chunks per softmax batch
NGRP = NTC // GRP

F32 = mybir.dt.float32
F32R = mybir.dt.float32r
BF16 = mybir.dt.bfloat16
SA_F32R = False  # score path dtype (False -> bf16)

_CACHE = {}
BUILD_CFG = dict()
import os as _os
if _os.environ.get("KCFG"):
    import json as _json
    BUILD_CFG.update(_json.loads(_os.environ["KCFG"]))


def build_nc(reps=1, grp=GRP, eb=4, tb=4, db=6, wb=4, sb=3,
             sa_f32r=None, tail_g=2, taper=True, split_tail=False,
             gp_t2=True, gp_dsum=True, gp_cast=True, gp_t1=True,
             qw_dve=False, mult_splits=2, t1_dve=2):
    if sa_f32r is None:
        sa_f32r = SA_F32R
    SDT = F32R if sa_f32r else BF16     # score-path storage/matmul dtype
    nc = bacc.Bacc("TRN2", target_bir_lowering=False, debug=False,
                   num_devices=N_CORES)

    # Per-core inputs (host-prepped layouts).
    # qTp[d, h, q] = query[q, h*64+d]
    qTp = nc.dram_tensor("qTp", [DH, H * QPC], SDT, kind="ExternalInput")
    # vT1[i, t] = value[t, i] for i<64; row 64 = ones
    vT1 = nc.dram_tensor("vT1", [DH + 1, TV], SDT, kind="ExternalInput")
    # vP[p, c, d] = value[c*128 + p, d]
    vP = nc.dram_tensor("vP", [TCH, NTC * DH], BF16, kind="ExternalInput")
    # wA[d, h, i] = conv_w[h*64+d, i, 0] (i<64); wA[d, h, 64] = conv_b[h*64+d]
    wA = nc.dram_tensor("wA", [DH, H * (DH + 1)], SDT, kind="ExternalInput")
    out = nc.dram_tensor("out", [QPC, D], F32, kind="ExternalOutput")

    ngrp = NTC // grp

    with tile.TileContext(nc) as tc:
        with (
            tc.tile_pool(name="consts", bufs=1) as consts,
            tc.tile_pool(name="qwt", bufs=1) as qwt_pool,
            tc.tile_pool(name="vals", bufs=1) as vals,
        ):
            # ---- constants / weights ----
            w_sb = consts.tile([DH, H, DH + 1], SDT)
            nc.sync.dma_start(out=w_sb, in_=wA.rearrange("d (h i) -> d h i", h=H))

            vT_sb = vals.tile([DH + 1, TV], SDT)
            nc.gpsimd.dma_start(out=vT_sb, in_=vT1[:, :])

            ident = consts.tile([2 * DH, 2 * DH], F32)
            make_identity(nc, ident)

            # ---- qWT' computation: [65, H, QPC] in two h-halves ----
            # per-h input DMAs spread over queues so qwt matmuls start early
            qTp3 = qTp.rearrange("d (h q) -> d h q", h=H)
            qwt_halves = []
            dma_engs = [nc.sync, nc.gpsimd, nc.scalar, nc.sync]
            with (
                tc.tile_pool(name="qt_in", bufs=2) as qt_in,
                tc.tile_pool(name="qw_ps", bufs=2, space="PSUM") as qw_ps,
            ):
                for half in (0, 1):
                    qt_sb = qt_in.tile([DH, 4, QPC], SDT)
                    for pair in (0, 1):
                        eng = dma_engs[(half * 2 + pair) % 4]
                        eng.dma_start(
                            out=qt_sb[:, pair * 2:pair * 2 + 2, :],
                            in_=qTp3[:, half * 4 + pair * 2:
                                     half * 4 + pair * 2 + 2, :])
                    qwt_h = qwt_pool.tile([DH + 1, 4, QPC], SDT,
                                          tag=f"qwt{half}")
                    for hh in range(4):
                        h = half * 4 + hh
                        ps = qw_ps.tile([DH + 1, QPC], F32)
                        nc.tensor.matmul(ps, lhsT=w_sb[:, h, :],
                                         rhs=qt_sb[:, hh, :],
                                         start=True, stop=True)
                        # alternate engines so the preamble copies pipeline
                        if (hh + half) % 2 == (0 if qw_dve else 1):
                            nc.vector.tensor_copy(qwt_h[:, hh, :], ps)
                        else:
                            nc.scalar.copy(qwt_h[:, hh, :], ps)
                    qwt_halves.append(qwt_h)

            v_sb = vals.tile([TCH, NTC, DH], BF16)
            nc.gpsimd.dma_start(out=v_sb,
                                in_=vP.rearrange("p (c d) -> p c d", c=NTC))

            # ---- main loop: flattened chunk pipeline ----
            with (
                tc.tile_pool(name="s_ps", bufs=sb, space="PSUM") as s_ps_pool,
                tc.tile_pool(name="ctx_ps", bufs=2, space="PSUM") as ctx_ps_pool,
                tc.tile_pool(name="e_sb", bufs=eb) as e_pool,
                tc.tile_pool(name="tr_sb", bufs=tb) as tr_pool,
                tc.tile_pool(name="d_sb", bufs=db) as d_pool,
                tc.tile_pool(name="w_sb2", bufs=wb) as wt_pool,
                tc.tile_pool(name="ctx_sb", bufs=2) as ctx_sb_pool,
            ):
                import contextlib
                rep_ctx = tc.For_i(0, reps, 1) if reps > 1 else contextlib.nullcontext()
                with rep_ctx:
                    ctx_tiles = {}

                    def emit_scores(qb, g, ci, e):
                        """score matmuls + exp for chunk (qb, g, ci)"""
                        q0 = qb * QB
                        t = g * grp + ci
                        s_ps = s_ps_pool.tile([TCH, H, QB], F32, tag="s")
                        lhsT = vT_sb[:, t * TCH:(t + 1) * TCH]
                        nc.tensor.matmul(
                            s_ps[:, 0:4, :], lhsT=lhsT,
                            rhs=qwt_halves[0][:, :, q0:q0 + QB],
                            start=True, stop=True)
                        nc.tensor.matmul(
                            s_ps[:, 4:8, :], lhsT=lhsT,
                            rhs=qwt_halves[1][:, :, q0:q0 + QB],
                            start=True, stop=True)
                        nc.scalar.activation(
                            out=e[:, ci, :, :], in_=s_ps,
                            func=mybir.ActivationFunctionType.Exp,
                            scale=0.125)
                        return s_ps

                    def emit_softmax(e, wt, cis):
                        """normalize chunks cis (a slice) of group tiles"""
                        t1 = tr_pool.tile([TCH, grp, 4, QB], BF16, tag="t1")
                        # split the first reduction level between DVE and Pool
                        k = t1_dve
                        if k:
                            nc.vector.tensor_add(
                                t1[:, cis, 0:k], e[:, cis, 0:k, :],
                                e[:, cis, 4:4 + k, :])
                        if k < 4:
                            (nc.gpsimd if gp_t1 else nc.vector).tensor_add(
                                t1[:, cis, k:4], e[:, cis, k:4, :],
                                e[:, cis, 4 + k:8, :])
                        t2 = tr_pool.tile([TCH, grp, 2, QB], BF16, tag="t2")
                        (nc.gpsimd if gp_t2 else nc.vector).tensor_add(
                            t2[:, cis], t1[:, cis, 0:2, :], t1[:, cis, 2:4, :])
                        dsum = d_pool.tile([TCH, grp, QB], F32, tag="dsum")
                        (nc.gpsimd if gp_dsum else nc.vector).tensor_add(
                            dsum[:, cis], t2[:, cis, 0, :], t2[:, cis, 1, :])
                        r32 = d_pool.tile([TCH, grp, QB], F32, tag="r32")
                        nc.vector.reciprocal_approx_fast(out=r32[:, cis],
                                                         in_=dsum[:, cis])
                        r16 = d_pool.tile([TCH, grp, QB], BF16, tag="r16")
                        (nc.gpsimd if gp_cast else nc.vector).tensor_copy(
                            r16[:, cis], r32[:, cis])
                        n = cis.stop - cis.start
                        nsp = mult_splits if n > 1 else 1
                        step = max(1, n // nsp)
                        for c0 in range(cis.start, cis.stop, step):
                            cs = slice(c0, min(c0 + step, cis.stop))
                            rs = r16[:, cs]
                            r_bcast = bass.AP(
                                tensor=rs.tensor, offset=rs.offset,
                                ap=[rs.ap[0], rs.ap[1], [0, H], rs.ap[2]],
                            )
                            nc.vector.tensor_mul(wt[:, cs], e[:, cs], r_bcast)

                    def emit_stage_b(qb, g, wt, cis):
                        ctx_ps = ctx_tiles[qb]
                        for ci in range(cis.start, cis.stop):
                            t = g * grp + ci
                            for sub in (0, 1):
                                nc.tensor.matmul(
                                    ctx_ps[sub * DH:(sub + 1) * DH, :, :],
                                    lhsT=v_sb[:, t, :],
                                    rhs=wt[:, ci, :,
                                           sub * (QB // 2):(sub + 1) * (QB // 2)],
                                    start=(t == 0), stop=(t == NTC - 1),
                                    tile_position=(0, sub * DH),
                                    skip_group_check=True)

                    def emit_tail(qb, last=False):
                        q0_ = qb * QB
                        ctx_ps = ctx_tiles.pop(qb)
                        ctx_sb = ctx_sb_pool.tile([2 * DH, H, QB // 2], F32)
                        subs = ((0,), (1,)) if split_tail else ((0, 1),)
                        o_ps = s_ps_pool.tile([DH, 2, H, DH], F32, tag="s")
                        o_sb = ctx_sb_pool.tile([DH, 2, H, DH], F32,
                                                tag="o_sb")

                        def cp(dst, src):
                            # the last tail runs after the final exp: ACT is
                            # idle there while DVE still drains the backlog
                            if last:
                                nc.scalar.copy(dst, src)
                            else:
                                nc.vector.tensor_copy(dst, src)

                        dma_q = (nc.sync, nc.gpsimd)
                        for subset in subs:
                            for sub in subset:
                                cp(ctx_sb[sub * DH:(sub + 1) * DH],
                                   ctx_ps[sub * DH:(sub + 1) * DH])
                            for sub in subset:
                                for h in range(H):
                                    nc.tensor.transpose(
                                        o_ps[:, sub, h, :],
                                        ctx_sb[sub * DH:(sub + 1) * DH, h, :],
                                        ident[sub * DH:(sub + 1) * DH,
                                              sub * DH:(sub + 1) * DH])
                            for sub in subset:
                                cp(o_sb[:, sub], o_ps[:, sub])
                                sub_ap = bass.AP(
                                    tensor=out, offset=q0_ * D + sub * DH * D,
                                    ap=[[D, DH], [1, D]],
                                )
                                dma_q[sub].dma_start(out=sub_ap,
                                                     in_=o_sb[:, sub])

                    # job list: (qb, g) pairs in order
                    jobs = [(qb, g) for qb in range(NQB) for g in range(ngrp)]
                    pending_b = None   # (qb, g, wt, slice) awaiting stage B
                    pending_tail = None
                    for j, (qb, g) in enumerate(jobs):
                        if qb not in ctx_tiles:
                            ctx_tiles[qb] = ctx_ps_pool.tile(
                                [2 * DH, H, QB // 2], F32, tag="ctx",
                                name=f"ctx{qb}")
                        e = e_pool.tile([TCH, grp, H, QB], BF16)
                        wt = wt_pool.tile([TCH, grp, H, QB], BF16)
                        last_job = j == len(jobs) - 1
                        if taper and last_job:
                            # fine-grained emission to shrink the serial tail
                            if pending_b is not None:
                                emit_stage_b(*pending_b)
                                pending_b = None
                            ts = taper if isinstance(taper, int) and taper > 1 else 1
                            for c0 in range(0, grp, ts):
                                cs = slice(c0, min(c0 + ts, grp))
                                for ci in range(cs.start, cs.stop):
                                    emit_scores(qb, g, ci, e)
                                emit_softmax(e, wt, cs)
                                emit_stage_b(qb, g, wt, cs)
                        else:
                            for ci in range(grp):
                                emit_scores(qb, g, ci, e)
                            if pending_tail is not None and g == tail_g:
                                emit_tail(pending_tail)
                                pending_tail = None
                            if pending_b is not None:
                                emit_stage_b(*pending_b)
                                pending_b = None
                            emit_softmax(e, wt, slice(0, grp))
                            pending_b = (qb, g, wt, slice(0, grp))
                        if g == ngrp - 1:
                            if pending_tail is not None:
                                emit_tail(pending_tail)
                            pending_tail = qb
                    if pending_b is not None:
                        emit_stage_b(*pending_b)
                    if pending_tail is not None:
                        emit_tail(pending_tail, last=True)
    nc.compile()
    return nc


def _prep_inputs(query, value, conv_w, conv_b):
    """Build the 8 per-core input maps (host-side sharding + layout)."""
    sdt = np.float32 if SA_F32R else ml_dtypes.bfloat16
    W = conv_w[:, :, 0]  # [512, 64]
    w_aug = np.zeros((DH, H, DH + 1), np.float32)
    w_aug[:, :, :DH] = W.reshape(H, DH, DH).transpose(1, 0, 2)
    w_aug[:, :, DH] = conv_b.reshape(H, DH).T
    w_aug = w_aug.reshape(DH, H * (DH + 1)).astype(sdt)

    per_batch = {}
    for b in range(B):
        vT1 = np.concatenate(
            [value[b].T, np.ones((1, TV), np.float32)], axis=0
        ).astype(sdt)
        vPb = np.ascontiguousarray(
            value[b].reshape(NTC, TCH, DH).transpose(1, 0, 2)
        ).reshape(TCH, NTC * DH).astype(ml_dtypes.bfloat16)
        per_batch[b] = (np.ascontiguousarray(vT1), np.ascontiguousarray(vPb))

    in_maps = []
    for c in range(N_CORES):
        b = c // (N_CORES // B)
        qs = (c % (N_CORES // B)) * QPC
        # qTp[d, h, q] = query[b, qs+q, h*64+d]
        qTp = np.ascontiguousarray(
            query[b, qs:qs + QPC, :].reshape(QPC, H, DH).transpose(2, 1, 0)
        ).reshape(DH, H * QPC).astype(sdt)
        vT1, vPb = per_batch[b]
        in_maps.append({
            "qTp": np.ascontiguousarray(qTp),
            "vT1": vT1,
            "vP": vPb,
            "wA": w_aug,
        })
    return in_maps


def kernel(query, value, conv_w, conv_b, trace=False, **bench_kwargs):
    query = np.asarray(query, np.float32)
    value = np.asarray(value, np.float32)
    conv_w = np.asarray(conv_w, np.float32)
    conv_b = np.asarray(conv_b, np.float32)

    if "nc" not in _CACHE:
        _CACHE["nc"] = build_nc(**BUILD_CFG)
    nc = _CACHE["nc"]

    in_maps = _prep_inputs(query, value, conv_w, conv_b)
    res = run_bass_kernel_spmd(nc, in_maps, core_ids=list(range(N_CORES)),
                               trace=trace, **bench_kwargs)

    out = np.empty((B, TQ, D), np.float32)
    for c in range(N_CORES):
        b = c // (N_CORES // B)
        qs = (c % (N_CORES // B)) * QPC
        out[b, qs:qs + QPC, :] = res.results[c]["out"]
    if trace:
        return out, res
    return out
